# revision 43
# baseline (speedup 1.0000x reference)
"""Self-contained Trainium2 Bass kernel for an 11-layer transformer LM forward
(B=2, S=1024, D=216, NH=4, HD=54, FF=864, V=32000) on 8 NeuronCores.

Sharding: data-parallel over batch (cores 0-3 batch 0, cores 4-7 batch 1);
within each batch group, the lm_head / logits are sharded 4 ways over vocab.
Each core runs the full trunk for its batch (activations transposed [D, S] on
chip, bf16 matmuls with f32 accumulation) and produces logitsT [8000, 1024]
bf16 for its vocab shard (upcast to f32 on host).
"""

import types

import numpy as np
import ml_dtypes

import concourse.bacc as bacc
import concourse.bass as bass
import concourse.mybir as mybir
import concourse.tile as tile
from concourse import bass_utils

# model dims (hardcoded per problem spec)
V, D, NL, NH, HD, FF = 32000, 216, 11, 4, 54, 864
B, S = 2, 1024
EPS = 1e-5
THETA = 10000.0
NCORES = 8
VSPLIT = 4
VS = V // VSPLIT  # 8000 vocab rows per core
DC = D // 2  # 108: d-dim partition chunk
FC = FF // 8  # 108: ff-dim chunk
HDH = HD // 2  # 27
HP = 32  # padded half-head partition block
SCALE = HD**-0.5
QB = 256  # attention query block
NQB = S // QB
NKC = S // 128
SH = 512  # s-half size for most matmuls
MASKVAL = -1.0e9

F32 = mybir.dt.float32
BF16 = mybir.dt.bfloat16
I32 = mybir.dt.int32
AF = mybir.ActivationFunctionType
OP = mybir.AluOpType

_NC_CACHE = None


def _patch_act_tables(nc):
    """Make ln/exp/copy resolve to one combined act-func set (keeping each
    set's original index) so the compiler doesn't thrash table loads between
    rmsnorm Ln/Exp and attention Exp. Falls back to default behavior if no
    combined set exists."""
    from concourse.hw_specs import get_activation_tables
    import bass_rust as _br

    def patched(self):
        has_activation = any(
            isinstance(i, mybir.InstActivation)
            for b in self.main_func.blocks
            for i in b.instructions
        )
        if not has_activation:
            return
        tables = list(get_activation_tables(self.m.arch).items())
        combined = None
        for name, s in tables:
            funcs = {str(f).split(".")[-1].lower() for f in s}
            if {"ln", "exp", "copy"} <= funcs:
                combined = name
                break
        if combined is not None:
            shadow = set()
            for f in next(s for n, s in tables if n == combined):
                fn = str(f).split(".")[-1].lower()
                if fn in ("ln", "exp", "copy", "identity"):
                    shadow.add(f)
            doctored = []
            for name, s in tables:
                if name == combined:
                    doctored.append((name, s))
                else:
                    doctored.append((name, s - shadow))
            tables = doctored
        _br.insert_act_table_loads(self, tables)

    nc.insert_act_table_loads = types.MethodType(patched, nc)


def build_nc():
    global _NC_CACHE
    if _NC_CACHE is not None:
        return _NC_CACHE
    nc = bacc.Bacc("TRN2", target_bir_lowering=False, debug=False)
    _patch_act_tables(nc)

    # ---- DRAM tensors (per-core inputs) ----
    ids_d = nc.dram_tensor("ids", [S, 1], I32, kind="ExternalInput")
    embed_d = nc.dram_tensor("embed", [V, D], F32, kind="ExternalInput")
    cos4_d = nc.dram_tensor("cos4", [128, S], BF16, kind="ExternalInput")
    sin4p_d = nc.dram_tensor("sin4p", [128, S], BF16, kind="ExternalInput")
    sin4n_d = nc.dram_tensor("sin4n", [128, S], BF16, kind="ExternalInput")
    maskAB_d = nc.dram_tensor("maskAB", [128, 2 * QB], BF16, kind="ExternalInput")
    idn_d = nc.dram_tensor("idn", [128, 128], F32, kind="ExternalInput")
    idnB_d = nc.dram_tensor("idnB", [128, 128], BF16, kind="ExternalInput")
    onesEr_d = nc.dram_tensor("onesEr", [1, 64], mybir.dt.float32r, kind="ExternalInput")
    # packed weights, see host prep below
    qkvw_d = nc.dram_tensor("qkvw", [NL, 2, DC, 768], BF16, kind="ExternalInput")
    ow_d = nc.dram_tensor("ow", [NL, 2, 128, D], BF16, kind="ExternalInput")
    gw_d = nc.dram_tensor("gw", [NL, 2, DC, FF], BF16, kind="ExternalInput")
    uw_d = nc.dram_tensor("uw", [NL, 2, DC, FF], BF16, kind="ExternalInput")
    dw_d = nc.dram_tensor("dw", [NL, 8, FC, D], BF16, kind="ExternalInput")
    lmh_d = nc.dram_tensor("lmh", [2, DC, VS], BF16, kind="ExternalInput")
    out_d = nc.dram_tensor("logitsT", [VS, S], BF16, kind="ExternalOutput")

    with tile.TileContext(nc) as tc:
        with (
            tc.tile_pool(name="cst", bufs=1) as cst,
            tc.tile_pool(name="xp", bufs=1) as xp,
            tc.tile_pool(name="wp", bufs=2) as wp,
            tc.tile_pool(name="hp", bufs=2) as hpool,
            tc.tile_pool(name="sqp", bufs=2) as sqp,
            tc.tile_pool(name="qk", bufs=2) as qkp,
            tc.tile_pool(name="vp", bufs=1) as vp,
            tc.tile_pool(name="atp", bufs=10) as atp,
            tc.tile_pool(name="aop", bufs=2) as aop,
            tc.tile_pool(name="tp", bufs=8) as tp,
            tc.tile_pool(name="sm", bufs=1) as sm,
            tc.tile_pool(name="rt", bufs=4) as rt,
            tc.tile_pool(name="bp", bufs=2) as bp,
            tc.tile_pool(name="lm", bufs=2) as lmp,
            tc.tile_pool(name="lo", bufs=2) as lop,
            tc.tile_pool(name="eb", bufs=2) as ebp,
            tc.tile_pool(name="ps", bufs=8, space="PSUM") as ps,
        ):
            # ---- constants ----
            cos4 = cst.tile([128, S], BF16, tag="cos4")
            sin4p = cst.tile([128, S], BF16, tag="sin4p")
            sin4n = cst.tile([128, S], BF16, tag="sin4n")
            maskAB = cst.tile([128, 2 * QB], BF16, tag="maskAB")
            idn = cst.tile([128, 128], F32, tag="idn")
            idnB = cst.tile([128, 128], BF16, tag="idnB")
            for t, d in (
                (cos4, cos4_d), (sin4p, sin4p_d), (sin4n, sin4n_d),
                (maskAB, maskAB_d), (idn, idn_d), (idnB, idnB_d),
            ):
                nc.sync.dma_start(out=t, in_=d.ap())
            ones108 = cst.tile([DC, 1], BF16, tag="ones108")
            nc.vector.memset(ones108, 1.0)
            onesE = cst.tile([1, DC], BF16, tag="onesE")
            nc.vector.memset(onesE, 1.0)
            ones1S = cst.tile([1, S], F32, tag="ones1S")
            nc.vector.memset(ones1S, 1.0)
            onesEr = cst.tile([1, 64], mybir.dt.float32r, tag="onesEr")
            nc.sync.dma_start(out=onesEr, in_=onesEr_d.ap())
            lnbias = cst.tile([1, 1], F32, tag="lnbias")
            nc.vector.memset(lnbias, EPS)

            # persistent v buffer: [128, kc, head*72]; per head slot cols
            # 0:54 hold v, 54:64 zero pad, 64 a column of ones (denominator
            # trick: attn@v row 64 = sum of exp) -- constants set once here.
            v_sb = cst.tile([128, NKC, NH * 72], BF16, tag="v_sb")
            nc.vector.memset(v_sb, 0.0)
            nc.vector.memset(
                v_sb.rearrange("p a (h c) -> p a h c", h=NH)[:, :, :, 64:65], 1.0
            )

            # ---- residual stream xT: two d-chunks [108, S] f32 ----
            x = [xp.tile([DC, S], F32, tag=f"x{dc}", name=f"x{dc}") for dc in range(2)]

            # ---- embedding gather + transpose ----
            ids_sb = ebp.tile([128, NKC], I32, tag="ids")
            nc.sync.dma_start(
                out=ids_sb, in_=ids_d.ap().rearrange("(c p) o -> p (c o)", p=128)
            )
            for sc in range(NKC):
                xg = ebp.tile([128, D], F32, tag="xg")
                nc.gpsimd.indirect_dma_start(
                    out=xg,
                    out_offset=None,
                    in_=embed_d.ap(),
                    in_offset=bass.IndirectOffsetOnAxis(ap=ids_sb[:, sc : sc + 1], axis=0),
                )
                for dc in range(2):
                    pt = ps.tile([DC, 128], F32, tag="ps")
                    nc.tensor.transpose(
                        out=pt, in_=xg[:, dc * DC : (dc + 1) * DC], identity=idn
                    )
                    nc.vector.tensor_copy(
                        out=x[dc][:, sc * 128 : (sc + 1) * 128], in_=pt
                    )

            def rms_partA(xt, sq, tagp, sh):
                """sq + mean-square reduce + Ln/Exp -> rstd tile."""
                sl = slice(sh * SH, (sh + 1) * SH)
                for dc in range(2):
                    nc.gpsimd.tensor_tensor(
                        out=sq[dc][:, sl], in0=xt[dc][:, sl], in1=xt[dc][:, sl],
                        op=OP.mult,
                    )
                ms = ps.tile([1, SH], F32, tag="ps", name=f"ms_{tagp}{sh}")
                for dc in range(2):
                    nc.tensor.matmul(
                        out=ms, lhsT=ones108, rhs=sq[dc][:, sl],
                        start=(dc == 0), stop=(dc == 1),
                    )
                lnt = sm.tile([1, SH], F32, tag="lnt", name=f"lnt_{tagp}{sh}", bufs=2)
                nc.scalar.activation(
                    out=lnt, in_=ms, func=AF.Ln, scale=1.0 / D, bias=lnbias
                )
                rstd = sm.tile([1, SH], BF16, tag="rstd", name=f"rstd_{tagp}{sh}", bufs=4)
                nc.scalar.activation(out=rstd, in_=lnt, func=AF.Exp, scale=-0.5)
                return rstd

            def rms_partB(xt, h, rstd, tagp, sh):
                """broadcast rstd + apply: h = x * rstd."""
                sl = slice(sh * SH, (sh + 1) * SH)
                bc = ps.tile([DC, SH], F32, tag="ps", name=f"bc_{tagp}{sh}")
                nc.tensor.matmul(out=bc, lhsT=onesE, rhs=rstd, start=True, stop=True)
                for dc in range(2):
                    nc.vector.tensor_tensor(
                        out=h[dc][:, sl], in0=xt[dc][:, sl], in1=bc, op=OP.mult
                    )

            def rms_half(xt, h, sq, tagp, sh):
                rstd = rms_partA(xt, sq, tagp, sh)
                rms_partB(xt, h, rstd, tagp, sh)

            def rms_alloc(tagp):
                h = [
                    hpool.tile([DC, S], BF16, tag=f"h{dc}", name=f"{tagp}{dc}")
                    for dc in range(2)
                ]
                sq = [
                    sqp.tile([DC, S], BF16, tag=f"sq{dc}", name=f"sq_{tagp}{dc}")
                    for dc in range(2)
                ]
                return h, sq

            def rmsnorm(xt, tagp):
                """xt: list of 2 [108, S] f32 chunks -> h (2 chunks, bf16)."""
                h, sq = rms_alloc(tagp)
                for sh in range(2):
                    rms_half(xt, h, sq, tagp, sh)
                return h

            def load_weights(l):
                qkvw = wp.tile([DC, 2, 768], BF16, tag="qkvw")
                nc.sync.dma_start(
                    out=qkvw, in_=qkvw_d.ap()[l].rearrange("a p c -> p a c")
                )
                ow = wp.tile([128, 2, D], BF16, tag="ow")
                nc.sync.dma_start(out=ow, in_=ow_d.ap()[l].rearrange("a p c -> p a c"))
                gw = wp.tile([DC, 2, FF], BF16, tag="gw")
                nc.sync.dma_start(out=gw, in_=gw_d.ap()[l].rearrange("a p c -> p a c"))
                uw = wp.tile([DC, 2, FF], BF16, tag="uw")
                nc.sync.dma_start(out=uw, in_=uw_d.ap()[l].rearrange("a p c -> p a c"))
                dw = wp.tile([FC, 8, D], BF16, tag="dw")
                nc.sync.dma_start(out=dw, in_=dw_d.ap()[l].rearrange("a p c -> p a c"))
                return dict(qkvw=qkvw, ow=ow, gw=gw, uw=uw, dw=dw)

            wts = load_weights(0)

            def emit_qkv_rope(l, h, qkvw, tiles):
                qA, qB, kA, kB = tiles
                for sh in range(2):
                    sl = slice(sh * SH, (sh + 1) * SH)
                    for (tA, tB, base) in ((qA, qB, 0), (kA, kB, 256)):
                        pA = ps.tile([128, SH], F32, tag="ps", name=f"pA{l}_{sh}_{base}")
                        pB = ps.tile([128, SH], F32, tag="ps", name=f"pB{l}_{sh}_{base}")
                        for dc in range(2):
                            nc.tensor.matmul(
                                out=pA, lhsT=qkvw[:, dc, base : base + 128],
                                rhs=h[dc][:, sl], start=(dc == 0), stop=(dc == 1),
                            )
                        for dc in range(2):
                            nc.tensor.matmul(
                                out=pB, lhsT=qkvw[:, dc, base + 128 : base + 256],
                                rhs=h[dc][:, sl], start=(dc == 0), stop=(dc == 1),
                            )
                        # rope: rotA = pA*cos + pB*(-sin); rotB = pB*cos + pA*sin
                        Bsb = bp.tile([128, SH], BF16, tag="Bsb", name=f"Bsb{l}_{sh}_{base}")
                        nc.scalar.copy(out=Bsb, in_=pB)
                        t1 = rt.tile([128, SH], BF16, tag="rt1")
                        nc.vector.tensor_tensor(out=t1, in0=pA, in1=cos4[:, sl], op=OP.mult)
                        t2 = rt.tile([128, SH], BF16, tag="rt2")
                        nc.gpsimd.tensor_tensor(out=t2, in0=Bsb, in1=sin4n[:, sl], op=OP.mult)
                        nc.vector.tensor_tensor(out=tA[:, sl], in0=t1, in1=t2, op=OP.add)
                        t3 = rt.tile([128, SH], BF16, tag="rt3")
                        nc.gpsimd.tensor_tensor(out=t3, in0=Bsb, in1=cos4[:, sl], op=OP.mult)
                        t4 = rt.tile([128, SH], BF16, tag="rt4")
                        nc.vector.tensor_tensor(out=t4, in0=pA, in1=sin4p[:, sl], op=OP.mult)
                        nc.vector.tensor_tensor(out=tB[:, sl], in0=t3, in1=t4, op=OP.add)
                    # v for this half's 4 k-chunks
                    for sc in range(sh * 4, sh * 4 + 4):
                        pv = ps.tile([128, 256], F32, tag="ps", name=f"pv{l}_{sc}")
                        for dc in range(2):
                            nc.tensor.matmul(
                                out=pv, lhsT=h[dc][:, sc * 128 : (sc + 1) * 128],
                                rhs=qkvw[:, dc, 512:768], start=(dc == 0), stop=(dc == 1),
                            )
                        nc.vector.tensor_copy(
                            out=v_sb[:, sc].rearrange("p (h c) -> p h c", h=NH)[:, :, 0:54],
                            in_=pv[:, 0:D].rearrange("p (h c) -> p h c", h=NH),
                        )

            def emit_scores(l, h4, qb, tiles):
                qA, qB, kA, kB = tiles
                rowsl = slice(HP * h4, HP * h4 + HP)
                tpos = (HP * h4, 0)
                qsl = slice(qb * QB, (qb + 1) * QB)
                ats = []
                for pr in range(qb + 1):
                    kc0 = 2 * pr
                    spair = ps.tile(
                        [128, 2 * QB], F32, tag="ps", name=f"sc{l}_{h4}_{qb}_{pr}"
                    )
                    for j in range(2):
                        kc = kc0 + j
                        ksl = slice(kc * 128, (kc + 1) * 128)
                        csl = slice(j * QB, (j + 1) * QB)
                        diag = pr == qb
                        nc.tensor.matmul(
                            out=spair[:, csl], lhsT=kA[rowsl, ksl],
                            rhs=qA[rowsl, qsl],
                            start=True, stop=False, tile_position=tpos,
                        )
                        nc.tensor.matmul(
                            out=spair[:, csl], lhsT=kB[rowsl, ksl],
                            rhs=qB[rowsl, qsl],
                            start=False, stop=not diag, tile_position=tpos,
                        )
                        if diag:
                            # causal mask: accumulate identity @ mask
                            nc.tensor.matmul(
                                out=spair[:, csl], lhsT=idnB,
                                rhs=maskAB[:, csl],
                                start=False, stop=True,
                                skip_group_check=True,
                            )
                    at = atp.tile(
                        [128, 2 * QB], BF16, tag="at", name=f"at{l}_{h4}_{qb}_{pr}"
                    )
                    nc.scalar.activation(out=at, in_=spair, func=AF.Exp, scale=SCALE)
                    ats.append(at)
                return ats

            def emit_attention(l, tiles, ao, post_sh0_hook, post_sh0_hook2):
                den = [
                    sm.tile([1, S], F32, tag=f"den{h4}", name=f"den{l}_{h4}")
                    for h4 in range(NH)
                ]
                recip = [
                    sm.tile([1, S], mybir.dt.float32r, tag=f"recip{h4}", name=f"recip{l}_{h4}")
                    for h4 in range(NH)
                ]
                pvp_stash = {}

                norm_queue = []

                def emit_avs(h4, qb, ats, gidx):
                    qsl = slice(qb * QB, (qb + 1) * QB)
                    nkc = 2 * (qb + 1)
                    pvp = ps.tile([128, QB], F32, tag="ps", name=f"pvp{l}_{h4}_{qb}")
                    for kc in range(nkc):
                        nc.tensor.matmul(
                            out=pvp[0:65, :],
                            lhsT=v_sb[:, kc, 72 * h4 : 72 * h4 + 65],
                            rhs=ats[kc // 2][:, (kc % 2) * QB : (kc % 2 + 1) * QB],
                            start=(kc == 0), stop=(kc == nkc - 1),
                        )
                    nc.scalar.copy(out=den[h4][:, qsl], in_=pvp[64:65, :])
                    pvp_stash[(h4, qb)] = pvp
                    if qb % 2 == 1:
                        sh = qb // 2
                        sl = slice(sh * SH, (sh + 1) * SH)
                        with nc.allow_low_precision(reason="recip f32r is full fp32"):
                            nc.vector.reciprocal(
                                out=recip[h4][:, sl], in_=den[h4][:, sl]
                            )
                        norm_queue.append((h4, sh, gidx))

                def emit_norm(h4, sh):
                    # bc/bcs/ao deferred one score-group after the divide so
                    # the PE never waits on the Pool-divide chain
                    sl = slice(sh * SH, (sh + 1) * SH)
                    bc = ps.tile([64, SH], F32, tag="ps", name=f"abc{l}_{h4}_{sh}")
                    nc.tensor.matmul(
                        out=bc, lhsT=onesEr, rhs=recip[h4][:, sl],
                        start=True, stop=True,
                    )
                    bcs = rt.tile([64, SH], BF16, tag="bcs", name=f"bcs{l}_{h4}_{sh}")
                    nc.vector.tensor_copy(out=bcs, in_=bc)
                    for qh in range(2):
                        qbb = 2 * sh + qh
                        nc.vector.tensor_tensor(
                            out=ao[h4 % 2][
                                64 * (h4 // 2) : 64 * (h4 // 2) + 64,
                                qbb * QB : (qbb + 1) * QB,
                            ],
                            in0=pvp_stash.pop((h4, qbb))[0:64, :],
                            in1=bcs[:, qh * QB : (qh + 1) * QB],
                            op=OP.mult,
                        )

                pend = None
                nsh0 = 0
                fired_at = None
                hook2_fired = False
                groups = [(h4, qb) for qb in range(NQB) for h4 in range(NH)]
                for gidx, (h4, qb) in enumerate(groups):
                    ats = emit_scores(l, h4, qb, tiles)
                    if pend is not None:
                        emit_avs(*pend, gidx)
                        while norm_queue and norm_queue[0][2] < gidx:
                            hh, ss, _ = norm_queue.pop(0)
                            emit_norm(hh, ss)
                            nsh0 += ss == 0
                    if nsh0 == NH and fired_at is None:
                        # all sh0 heads normalized: o-proj(sh0) overlaps the
                        # remaining big score groups
                        fired_at = gidx
                        post_sh0_hook()
                    elif fired_at is not None and gidx > fired_at and not hook2_fired:
                        # one group later: rms2(sh0) reduce (its residual/sq
                        # deps have drained behind the intervening scores)
                        hook2_fired = True
                        post_sh0_hook2()
                    pend = (h4, qb, ats)
                emit_avs(*pend, len(groups))
                while norm_queue:
                    hh, ss, _ = norm_queue.pop(0)
                    emit_norm(hh, ss)
                if fired_at is None:
                    post_sh0_hook()
                if not hook2_fired:
                    post_sh0_hook2()

            def oproj_half(l, ao, ow, sh):
                sl = slice(sh * SH, (sh + 1) * SH)
                for dco in range(2):
                    po = ps.tile([DC, SH], F32, tag="ps", name=f"po{l}_{sh}_{dco}")
                    for dci in range(2):
                        nc.tensor.matmul(
                            out=po, lhsT=ow[:, dci, dco * DC : (dco + 1) * DC],
                            rhs=ao[dci][:, sl], start=(dci == 0), stop=(dci == 1),
                        )
                    nc.vector.tensor_tensor(
                        out=x[dco][:, sl], in0=x[dco][:, sl], in1=po, op=OP.add
                    )

            def mlp_half(l, h2, tt, gw, uw, dw, sh, mid_hook=None):
                sl = slice(sh * SH, (sh + 1) * SH)
                for fc in range(8):
                    pg = ps.tile([FC, SH], F32, tag="ps", name=f"pg{l}_{sh}_{fc}")
                    pu = ps.tile([FC, SH], F32, tag="ps", name=f"pu{l}_{sh}_{fc}")
                    for dc in range(2):
                        nc.tensor.matmul(
                            out=pg, lhsT=gw[:, dc, fc * FC : (fc + 1) * FC],
                            rhs=h2[dc][:, sl], start=(dc == 0), stop=(dc == 1),
                        )
                    for dc in range(2):
                        nc.tensor.matmul(
                            out=pu, lhsT=uw[:, dc, fc * FC : (fc + 1) * FC],
                            rhs=h2[dc][:, sl], start=(dc == 0), stop=(dc == 1),
                        )
                    sg = rt.tile([FC, SH], BF16, tag="sg")
                    nc.scalar.activation(out=sg, in_=pg, func=AF.Silu, scale=1.0)
                    nc.vector.tensor_tensor(
                        out=tt[fc][:, sl], in0=sg, in1=pu, op=OP.mult
                    )
                if mid_hook is not None:
                    mid_hook()
                for dco in range(2):
                    pd = ps.tile([DC, SH], F32, tag="ps", name=f"pd{l}_{sh}_{dco}")
                    for fc in range(8):
                        nc.tensor.matmul(
                            out=pd, lhsT=dw[:, fc, dco * DC : (dco + 1) * DC],
                            rhs=tt[fc][:, sl], start=(fc == 0), stop=(fc == 7),
                        )
                    nc.vector.tensor_tensor(
                        out=x[dco][:, sl], in0=x[dco][:, sl], in1=pd, op=OP.add
                    )

            # =================== layers ===================
            # software-pipelined: rms1 of layer l+1 is emitted inside the
            # MLP of layer l (after each s-half's residual lands) so its
            # Act-engine chain hides behind MLP matmuls.
            h_cur, sq_cur = rms_alloc("h0_")
            for sh in range(2):
                rms_half(x, h_cur, sq_cur, "h0_", sh)
            pend_rmsB = []  # deferred part-B of the next layer's rms1
            for l in range(NL):
                for args in pend_rmsB:
                    rms_partB(*args)
                pend_rmsB = []
                h = h_cur
                tiles = (
                    qkp.tile([128, S], BF16, tag="qA", name=f"qA{l}"),
                    qkp.tile([128, S], BF16, tag="qB", name=f"qB{l}"),
                    qkp.tile([128, S], BF16, tag="kA", name=f"kA{l}"),
                    qkp.tile([128, S], BF16, tag="kB", name=f"kB{l}"),
                )
                emit_qkv_rope(l, h, wts["qkvw"], tiles)
                # o-proj(sh0) + rms2(sh0) are emitted inside the attention
                # stream (after all sh0 heads normalize) to hide their
                # latency behind the remaining score groups.
                h2, sq2 = rms_alloc(f"g{l}_")
                ow = wts["ow"]
                ao = [
                    aop.tile([128, S], BF16, tag=f"ao{c}", name=f"ao{l}_{c}")
                    for c in range(2)
                ]

                box = {}

                def post_sh0():
                    oproj_half(l, ao, ow, 0)

                def post_sh0b():
                    box["g0"] = rms_partA(x, sq2, f"g{l}_", 0)

                emit_attention(l, tiles, ao, post_sh0, post_sh0b)
                rms_partB(x, h2, box["g0"], f"g{l}_", 0)
                gw, uw, dw = wts["gw"], wts["uw"], wts["dw"]
                # prefetch next layer's weights (overlaps MLP compute)
                if l + 1 < NL:
                    wts = load_weights(l + 1)
                ntag = f"h{l + 1}_" if l + 1 < NL else "f_"
                h_nxt, sq_nxt = rms_alloc(ntag)
                tt = [
                    tp.tile([FC, S], BF16, tag="t", name=f"t{l}_{i}")
                    for i in range(8)
                ]
                def a_n0():
                    box["n0"] = rms_partA(x, sq_nxt, ntag, 0)

                # o-proj(sh1) hides behind mlp(sh0)'s gate/up matmuls
                mlp_half(
                    l, h2, tt, gw, uw, dw, 0,
                    mid_hook=lambda: oproj_half(l, ao, ow, 1),
                )
                rstd_g1 = rms_partA(x, sq2, f"g{l}_", 1)
                rms_partB(x, h2, rstd_g1, f"g{l}_", 1)
                mlp_half(l, h2, tt, gw, uw, dw, 1, mid_hook=a_n0)
                rstd_n1 = rms_partA(x, sq_nxt, ntag, 1)
                pend_rmsB = [
                    (x, h_nxt, box["n0"], ntag, 0),
                    (x, h_nxt, rstd_n1, ntag, 1),
                ]
                h_cur = h_nxt

            def emit_lm_head(hf):
                vg_sizes = [512] * 15 + [320]
                voff = 0
                use_act = False
                for vg, vgs in enumerate(vg_sizes):
                    lmw = lmp.tile([DC, 2, 512], BF16, tag="lmw")
                    nc.sync.dma_start(
                        out=lmw[:, :, 0:vgs],
                        in_=lmh_d.ap()[:, :, voff : voff + vgs].rearrange("a p c -> p a c"),
                    )
                    nvc = (vgs + 127) // 128
                    lo = lop.tile([128, 4, S], BF16, tag="lo")
                    for vc in range(nvc):
                        m = min(128, vgs - vc * 128)
                        for sh in range(2):
                            sl = slice(sh * SH, (sh + 1) * SH)
                            pl = ps.tile([128, SH], F32, tag="ps")
                            for dc in range(2):
                                nc.tensor.matmul(
                                    out=pl[0:m, :],
                                    lhsT=lmw[:, dc, vc * 128 : vc * 128 + m],
                                    rhs=hf[dc][:, sl], start=(dc == 0), stop=(dc == 1),
                                )
                            if use_act:
                                nc.scalar.copy(out=lo[0:m, vc, sl], in_=pl[0:m, :])
                            else:
                                nc.vector.tensor_copy(out=lo[0:m, vc, sl], in_=pl[0:m, :])
                            use_act = not use_act
                    # one batched store per vocab group (vgs rows)
                    nfull = vgs // 128
                    nc.sync.dma_start(
                        out=out_d.ap()[voff : voff + nfull * 128, :].rearrange(
                            "(a p) s -> p a s", p=128
                        ),
                        in_=lo[:, 0:nfull, :],
                    )
                    if vgs % 128:
                        nc.sync.dma_start(
                            out=out_d.ap()[voff + nfull * 128 : voff + vgs, :],
                            in_=lo[0 : vgs % 128, nfull, :],
                        )
                    voff += vgs

            # =================== final norm + lm head ===================
            for args in pend_rmsB:
                rms_partB(*args)
            emit_lm_head(h_cur)

    nc.compile()
    _NC_CACHE = nc
    return nc


def _host_prep(inputs):
    """Build per-core in_maps from full inputs."""
    input_ids = np.asarray(inputs["input_ids"])
    embed = np.asarray(inputs["embed"], dtype=np.float32)
    ln1_w = np.asarray(inputs["ln1_w"], dtype=np.float32)
    qkv_w = np.asarray(inputs["qkv_w"], dtype=np.float32)
    o_w = np.asarray(inputs["o_w"], dtype=np.float32)
    ln2_w = np.asarray(inputs["ln2_w"], dtype=np.float32)
    gate_w = np.asarray(inputs["gate_w"], dtype=np.float32)
    up_w = np.asarray(inputs["up_w"], dtype=np.float32)
    down_w = np.asarray(inputs["down_w"], dtype=np.float32)
    norm_w = np.asarray(inputs["norm_w"], dtype=np.float32)
    lm_head_w = np.asarray(inputs["lm_head_w"], dtype=np.float32)
    bf16 = ml_dtypes.bfloat16

    # rope tables (transposed, padded 27->32, tiled x4 heads)
    inv_freq = 1.0 / (THETA ** (np.arange(0, HD, 2, dtype=np.float32) / HD))  # [27]
    t = np.arange(S, dtype=np.float32)
    freqs = np.outer(inv_freq, t)  # [27, S]
    cosh = np.cos(freqs).astype(np.float32)
    sinh = np.sin(freqs).astype(np.float32)

    def pad_tile(a):  # [27, S] -> [128, S]
        z = np.zeros((HP, S), np.float32)
        z[:HDH] = a
        return np.tile(z, (NH, 1))

    cos4 = pad_tile(cosh).astype(bf16)
    sin4p = pad_tile(sinh).astype(bf16)
    sin4n = pad_tile(-sinh).astype(bf16)

    # causal mask bias tiles [128 k, 256 q]: allowed k <= q
    r = np.arange(128)[:, None]
    c = np.arange(QB)[None, :]
    maskA = np.where(r <= c, 0.0, MASKVAL).astype(np.float32)
    maskB = np.where(r <= c - 128, 0.0, MASKVAL).astype(np.float32)
    maskAB = np.concatenate([maskA, maskB], axis=1).astype(bf16)

    idn = np.eye(128, dtype=np.float32)
    idnB = np.eye(128, dtype=np.float32).astype(bf16)

    # ---- weight packing ----
    # fold ln weights into qkv/gate/up; norm into lm_head
    qkvT = (qkv_w * ln1_w[:, None, :]).transpose(0, 2, 1)  # [NL, D(d), 3D(e)]
    gwT = (gate_w * ln2_w[:, None, :]).transpose(0, 2, 1)  # [NL, D, FF]
    uwT = (up_w * ln2_w[:, None, :]).transpose(0, 2, 1)  # [NL, D, FF]
    owT = o_w.transpose(0, 2, 1)
    dwT = down_w.transpose(0, 2, 1)  # [NL, FF, D]
    lmT = (lm_head_w * norm_w[None, :]).T  # [D, V]

    # q/k packed with lo/hi split, 32-padded: cols [qA(128) qB(128) kA kB v(256)]
    qkvw_packed = np.zeros((NL, D, 768), np.float32)
    for h in range(NH):
        qkvw_packed[:, :, HP * h : HP * h + HDH] = qkvT[:, :, 54 * h : 54 * h + HDH]
        qkvw_packed[:, :, 128 + HP * h : 128 + HP * h + HDH] = qkvT[:, :, 54 * h + HDH : 54 * h + HD]
        qkvw_packed[:, :, 256 + HP * h : 256 + HP * h + HDH] = qkvT[:, :, D + 54 * h : D + 54 * h + HDH]
        qkvw_packed[:, :, 384 + HP * h : 384 + HP * h + HDH] = qkvT[:, :, D + 54 * h + HDH : D + 54 * h + HD]
    qkvw_packed[:, :, 512 : 512 + D] = qkvT[:, :, 2 * D : 3 * D]  # v natural
    qkvw = qkvw_packed.reshape(NL, 2, DC, 768).astype(bf16)

    ow = np.zeros((NL, 2, 128, D), np.float32)
    for hh in range(NH):
        cc, j = hh % 2, hh // 2
        ow[:, cc, 64 * j : 64 * j + HD, :] = owT[:, 54 * hh : 54 * (hh + 1), :]
    ow = ow.astype(bf16)
    gwp = gwT.reshape(NL, 2, DC, FF).astype(bf16)
    uwp = uwT.reshape(NL, 2, DC, FF).astype(bf16)
    dwp = dwT.reshape(NL, 8, FC, D).astype(bf16)

    common = dict(
        embed=embed, cos4=cos4, sin4p=sin4p, sin4n=sin4n,
        maskAB=maskAB, idn=idn, idnB=idnB,
        onesEr=np.ones((1, 64), np.float32),
        qkvw=qkvw, ow=ow, gw=gwp, uw=uwp, dw=dwp,
    )
    in_maps = []
    for core in range(NCORES):
        b = core // VSPLIT
        vs = core % VSPLIT
        m = dict(common)
        m["ids"] = input_ids[b].astype(np.int32).reshape(S, 1)
        m["lmh"] = np.ascontiguousarray(
            lmT[:, vs * VS : (vs + 1) * VS].reshape(2, DC, VS)
        ).astype(bf16)
        in_maps.append(m)
    return in_maps


def kernel(**inputs) -> np.ndarray:
    nc = build_nc()
    in_maps = _host_prep(inputs)
    res = bass_utils.run_bass_kernel_spmd(nc, in_maps, core_ids=list(range(NCORES)))
    out = np.empty((B, S, V), np.float32)
    for core in range(NCORES):
        b = core // VSPLIT
        vs = core % VSPLIT
        out[b, :, vs * VS : (vs + 1) * VS] = (
            np.asarray(res.results[core]["logitsT"]).astype(np.float32).T
        )
    return out


# revision 45
# speedup vs baseline: 1.0165x; 1.0165x over previous
"""Self-contained Trainium2 Bass kernel for an 11-layer transformer LM forward
(B=2, S=1024, D=216, NH=4, HD=54, FF=864, V=32000) on 8 NeuronCores.

Sharding: data-parallel over batch (cores 0-3 batch 0, cores 4-7 batch 1);
within each batch group, the lm_head / logits are sharded 4 ways over vocab.
Each core runs the full trunk for its batch (activations transposed [D, S] on
chip, bf16 matmuls with f32 accumulation) and produces logitsT [8000, 1024]
bf16 for its vocab shard (upcast to f32 on host).
"""

import types

import numpy as np
import ml_dtypes

import concourse.bacc as bacc
import concourse.bass as bass
import concourse.mybir as mybir
import concourse.tile as tile
from concourse import bass_utils

# model dims (hardcoded per problem spec)
V, D, NL, NH, HD, FF = 32000, 216, 11, 4, 54, 864
B, S = 2, 1024
EPS = 1e-5
THETA = 10000.0
NCORES = 8
VSPLIT = 4
VS = V // VSPLIT  # 8000 vocab rows per core
DC = D // 2  # 108: d-dim partition chunk
FC = FF // 8  # 108: ff-dim chunk
HDH = HD // 2  # 27
HP = 32  # padded half-head partition block
SCALE = HD**-0.5
QB = 256  # attention query block
NQB = S // QB
NKC = S // 128
SH = 512  # s-half size for most matmuls
MASKVAL = -1.0e9

F32 = mybir.dt.float32
BF16 = mybir.dt.bfloat16
I32 = mybir.dt.int32
AF = mybir.ActivationFunctionType
OP = mybir.AluOpType

_NC_CACHE = None


def _patch_act_tables(nc):
    """Make ln/exp/copy resolve to one combined act-func set (keeping each
    set's original index) so the compiler doesn't thrash table loads between
    rmsnorm Ln/Exp and attention Exp. Falls back to default behavior if no
    combined set exists."""
    from concourse.hw_specs import get_activation_tables
    import bass_rust as _br

    def patched(self):
        has_activation = any(
            isinstance(i, mybir.InstActivation)
            for b in self.main_func.blocks
            for i in b.instructions
        )
        if not has_activation:
            return
        tables = list(get_activation_tables(self.m.arch).items())
        combined = None
        for name, s in tables:
            funcs = {str(f).split(".")[-1].lower() for f in s}
            if {"ln", "exp", "copy"} <= funcs:
                combined = name
                break
        if combined is not None:
            shadow = set()
            for f in next(s for n, s in tables if n == combined):
                fn = str(f).split(".")[-1].lower()
                if fn in ("ln", "exp", "copy", "identity"):
                    shadow.add(f)
            doctored = []
            for name, s in tables:
                if name == combined:
                    doctored.append((name, s))
                else:
                    doctored.append((name, s - shadow))
            tables = doctored
        _br.insert_act_table_loads(self, tables)

    nc.insert_act_table_loads = types.MethodType(patched, nc)


def build_nc():
    global _NC_CACHE
    if _NC_CACHE is not None:
        return _NC_CACHE
    nc = bacc.Bacc("TRN2", target_bir_lowering=False, debug=False)
    _patch_act_tables(nc)

    # ---- DRAM tensors (per-core inputs) ----
    ids_d = nc.dram_tensor("ids", [S, 1], I32, kind="ExternalInput")
    embed_d = nc.dram_tensor("embed", [V, D], F32, kind="ExternalInput")
    cos4_d = nc.dram_tensor("cos4", [128, S], BF16, kind="ExternalInput")
    sin4p_d = nc.dram_tensor("sin4p", [128, S], BF16, kind="ExternalInput")
    sin4n_d = nc.dram_tensor("sin4n", [128, S], BF16, kind="ExternalInput")
    maskAB_d = nc.dram_tensor("maskAB", [128, 2 * QB], BF16, kind="ExternalInput")
    idn_d = nc.dram_tensor("idn", [128, 128], F32, kind="ExternalInput")
    idnB_d = nc.dram_tensor("idnB", [128, 128], BF16, kind="ExternalInput")
    onesEr_d = nc.dram_tensor("onesEr", [1, 64], mybir.dt.float32r, kind="ExternalInput")
    # packed weights, see host prep below
    qkvw_d = nc.dram_tensor("qkvw", [NL, 2, DC, 768], BF16, kind="ExternalInput")
    ow_d = nc.dram_tensor("ow", [NL, 2, 128, D], BF16, kind="ExternalInput")
    gw_d = nc.dram_tensor("gw", [NL, 2, DC, FF], BF16, kind="ExternalInput")
    uw_d = nc.dram_tensor("uw", [NL, 2, DC, FF], BF16, kind="ExternalInput")
    dw_d = nc.dram_tensor("dw", [NL, 8, FC, D], BF16, kind="ExternalInput")
    lmh_d = nc.dram_tensor("lmh", [2, DC, VS], BF16, kind="ExternalInput")
    out_d = nc.dram_tensor("logitsT", [VS, S], BF16, kind="ExternalOutput")

    with tile.TileContext(nc) as tc:
        with (
            tc.tile_pool(name="cst", bufs=1) as cst,
            tc.tile_pool(name="xp", bufs=1) as xp,
            tc.tile_pool(name="wp", bufs=2) as wp,
            tc.tile_pool(name="hp", bufs=2) as hpool,
            tc.tile_pool(name="sqp", bufs=2) as sqp,
            tc.tile_pool(name="qk", bufs=2) as qkp,
            tc.tile_pool(name="vp", bufs=1) as vp,
            tc.tile_pool(name="atp", bufs=10) as atp,
            tc.tile_pool(name="aop", bufs=2) as aop,
            tc.tile_pool(name="tp", bufs=8) as tp,
            tc.tile_pool(name="sm", bufs=1) as sm,
            tc.tile_pool(name="rt", bufs=4) as rt,
            tc.tile_pool(name="bp", bufs=2) as bp,
            tc.tile_pool(name="lm", bufs=2) as lmp,
            tc.tile_pool(name="lo", bufs=2) as lop,
            tc.tile_pool(name="eb", bufs=2) as ebp,
            tc.tile_pool(name="ps", bufs=8, space="PSUM") as ps,
        ):
            # ---- constants ----
            cos4 = cst.tile([128, S], BF16, tag="cos4")
            sin4p = cst.tile([128, S], BF16, tag="sin4p")
            sin4n = cst.tile([128, S], BF16, tag="sin4n")
            maskAB = cst.tile([128, 2 * QB], BF16, tag="maskAB")
            idn = cst.tile([128, 128], F32, tag="idn")
            idnB = cst.tile([128, 128], BF16, tag="idnB")
            for t, d in (
                (cos4, cos4_d), (sin4p, sin4p_d), (sin4n, sin4n_d),
                (maskAB, maskAB_d), (idn, idn_d), (idnB, idnB_d),
            ):
                nc.sync.dma_start(out=t, in_=d.ap())
            ones108 = cst.tile([DC, 1], BF16, tag="ones108")
            nc.vector.memset(ones108, 1.0)
            onesE = cst.tile([1, DC], BF16, tag="onesE")
            nc.vector.memset(onesE, 1.0)
            ones1S = cst.tile([1, S], F32, tag="ones1S")
            nc.vector.memset(ones1S, 1.0)
            onesEr = cst.tile([1, 64], mybir.dt.float32r, tag="onesEr")
            nc.sync.dma_start(out=onesEr, in_=onesEr_d.ap())
            lnbias = cst.tile([1, 1], F32, tag="lnbias")
            nc.vector.memset(lnbias, EPS)

            # persistent v buffer: [128, kc, head*72]; per head slot cols
            # 0:54 hold v, 54:64 zero pad, 64 a column of ones (denominator
            # trick: attn@v row 64 = sum of exp) -- constants set once here.
            v_sb = cst.tile([128, NKC, NH * 72], BF16, tag="v_sb")
            nc.vector.memset(v_sb, 0.0)
            nc.vector.memset(
                v_sb.rearrange("p a (h c) -> p a h c", h=NH)[:, :, :, 64:65], 1.0
            )

            # ---- residual stream xT: two d-chunks [108, S] f32 ----
            x = [xp.tile([DC, S], F32, tag=f"x{dc}", name=f"x{dc}") for dc in range(2)]

            # ---- embedding gather + transpose ----
            ids_sb = ebp.tile([128, NKC], I32, tag="ids")
            nc.sync.dma_start(
                out=ids_sb, in_=ids_d.ap().rearrange("(c p) o -> p (c o)", p=128)
            )
            for sc in range(NKC):
                xg = ebp.tile([128, D], F32, tag="xg")
                nc.gpsimd.indirect_dma_start(
                    out=xg,
                    out_offset=None,
                    in_=embed_d.ap(),
                    in_offset=bass.IndirectOffsetOnAxis(ap=ids_sb[:, sc : sc + 1], axis=0),
                )
                for dc in range(2):
                    pt = ps.tile([DC, 128], F32, tag="ps")
                    nc.tensor.transpose(
                        out=pt, in_=xg[:, dc * DC : (dc + 1) * DC], identity=idn
                    )
                    nc.vector.tensor_copy(
                        out=x[dc][:, sc * 128 : (sc + 1) * 128], in_=pt
                    )

            def rms_partA(xt, sq, tagp, sh):
                """sq + mean-square reduce + Ln/Exp -> rstd tile."""
                sl = slice(sh * SH, (sh + 1) * SH)
                for dc in range(2):
                    nc.gpsimd.tensor_tensor(
                        out=sq[dc][:, sl], in0=xt[dc][:, sl], in1=xt[dc][:, sl],
                        op=OP.mult,
                    )
                ms = ps.tile([1, SH], F32, tag="ps", name=f"ms_{tagp}{sh}")
                for dc in range(2):
                    nc.tensor.matmul(
                        out=ms, lhsT=ones108, rhs=sq[dc][:, sl],
                        start=(dc == 0), stop=(dc == 1),
                    )
                lnt = sm.tile([1, SH], F32, tag="lnt", name=f"lnt_{tagp}{sh}", bufs=2)
                nc.scalar.activation(
                    out=lnt, in_=ms, func=AF.Ln, scale=1.0 / D, bias=lnbias
                )
                rstd = sm.tile([1, SH], BF16, tag="rstd", name=f"rstd_{tagp}{sh}", bufs=4)
                nc.scalar.activation(out=rstd, in_=lnt, func=AF.Exp, scale=-0.5)
                return rstd

            def rms_partB(xt, h, rstd, tagp, sh):
                """broadcast rstd + apply: h = x * rstd."""
                sl = slice(sh * SH, (sh + 1) * SH)
                bc = ps.tile([DC, SH], F32, tag="ps", name=f"bc_{tagp}{sh}")
                nc.tensor.matmul(out=bc, lhsT=onesE, rhs=rstd, start=True, stop=True)
                for dc in range(2):
                    nc.vector.tensor_tensor(
                        out=h[dc][:, sl], in0=xt[dc][:, sl], in1=bc, op=OP.mult
                    )

            def rms_half(xt, h, sq, tagp, sh):
                rstd = rms_partA(xt, sq, tagp, sh)
                rms_partB(xt, h, rstd, tagp, sh)

            def rms_alloc(tagp):
                h = [
                    hpool.tile([DC, S], BF16, tag=f"h{dc}", name=f"{tagp}{dc}")
                    for dc in range(2)
                ]
                sq = [
                    sqp.tile([DC, S], BF16, tag=f"sq{dc}", name=f"sq_{tagp}{dc}")
                    for dc in range(2)
                ]
                return h, sq

            def rmsnorm(xt, tagp):
                """xt: list of 2 [108, S] f32 chunks -> h (2 chunks, bf16)."""
                h, sq = rms_alloc(tagp)
                for sh in range(2):
                    rms_half(xt, h, sq, tagp, sh)
                return h

            def load_weights(l):
                qkvw = wp.tile([DC, 2, 768], BF16, tag="qkvw")
                nc.sync.dma_start(
                    out=qkvw, in_=qkvw_d.ap()[l].rearrange("a p c -> p a c")
                )
                ow = wp.tile([128, 2, D], BF16, tag="ow")
                nc.sync.dma_start(out=ow, in_=ow_d.ap()[l].rearrange("a p c -> p a c"))
                gw = wp.tile([DC, 2, FF], BF16, tag="gw")
                nc.sync.dma_start(out=gw, in_=gw_d.ap()[l].rearrange("a p c -> p a c"))
                uw = wp.tile([DC, 2, FF], BF16, tag="uw")
                nc.sync.dma_start(out=uw, in_=uw_d.ap()[l].rearrange("a p c -> p a c"))
                dw = wp.tile([FC, 8, D], BF16, tag="dw")
                nc.sync.dma_start(out=dw, in_=dw_d.ap()[l].rearrange("a p c -> p a c"))
                return dict(qkvw=qkvw, ow=ow, gw=gw, uw=uw, dw=dw)

            wts = load_weights(0)

            def emit_qkv_rope(l, h, qkvw, tiles):
                qA, qB, kA, kB = tiles
                for sh in range(2):
                    sl = slice(sh * SH, (sh + 1) * SH)
                    for (tA, tB, base) in ((qA, qB, 0), (kA, kB, 256)):
                        pA = ps.tile([128, SH], F32, tag="ps", name=f"pA{l}_{sh}_{base}")
                        pB = ps.tile([128, SH], F32, tag="ps", name=f"pB{l}_{sh}_{base}")
                        for dc in range(2):
                            nc.tensor.matmul(
                                out=pA, lhsT=qkvw[:, dc, base : base + 128],
                                rhs=h[dc][:, sl], start=(dc == 0), stop=(dc == 1),
                            )
                        for dc in range(2):
                            nc.tensor.matmul(
                                out=pB, lhsT=qkvw[:, dc, base + 128 : base + 256],
                                rhs=h[dc][:, sl], start=(dc == 0), stop=(dc == 1),
                            )
                        # rope: rotA = pA*cos + pB*(-sin); rotB = pB*cos + pA*sin
                        Bsb = bp.tile([128, SH], BF16, tag="Bsb", name=f"Bsb{l}_{sh}_{base}")
                        nc.scalar.copy(out=Bsb, in_=pB)
                        t1 = rt.tile([128, SH], BF16, tag="rt1")
                        nc.vector.tensor_tensor(out=t1, in0=pA, in1=cos4[:, sl], op=OP.mult)
                        t2 = rt.tile([128, SH], BF16, tag="rt2")
                        nc.gpsimd.tensor_tensor(out=t2, in0=Bsb, in1=sin4n[:, sl], op=OP.mult)
                        nc.vector.tensor_tensor(out=tA[:, sl], in0=t1, in1=t2, op=OP.add)
                        t3 = rt.tile([128, SH], BF16, tag="rt3")
                        nc.gpsimd.tensor_tensor(out=t3, in0=Bsb, in1=cos4[:, sl], op=OP.mult)
                        t4 = rt.tile([128, SH], BF16, tag="rt4")
                        nc.vector.tensor_tensor(out=t4, in0=pA, in1=sin4p[:, sl], op=OP.mult)
                        nc.vector.tensor_tensor(out=tB[:, sl], in0=t3, in1=t4, op=OP.add)
                    # v for this half's 4 k-chunks
                    for sc in range(sh * 4, sh * 4 + 4):
                        pv = ps.tile([128, 256], F32, tag="ps", name=f"pv{l}_{sc}")
                        for dc in range(2):
                            nc.tensor.matmul(
                                out=pv, lhsT=h[dc][:, sc * 128 : (sc + 1) * 128],
                                rhs=qkvw[:, dc, 512:768], start=(dc == 0), stop=(dc == 1),
                            )
                        nc.vector.tensor_copy(
                            out=v_sb[:, sc].rearrange("p (h c) -> p h c", h=NH)[:, :, 0:54],
                            in_=pv[:, 0:D].rearrange("p (h c) -> p h c", h=NH),
                        )

            def emit_scores(l, h4, qb, tiles):
                qA, qB, kA, kB = tiles
                rowsl = slice(HP * h4, HP * h4 + HP)
                tpos = (HP * h4, 0)
                qsl = slice(qb * QB, (qb + 1) * QB)
                ats = []
                for pr in range(qb + 1):
                    kc0 = 2 * pr
                    spair = ps.tile(
                        [128, 2 * QB], F32, tag="ps", name=f"sc{l}_{h4}_{qb}_{pr}"
                    )
                    for j in range(2):
                        kc = kc0 + j
                        ksl = slice(kc * 128, (kc + 1) * 128)
                        csl = slice(j * QB, (j + 1) * QB)
                        diag = pr == qb
                        nc.tensor.matmul(
                            out=spair[:, csl], lhsT=kA[rowsl, ksl],
                            rhs=qA[rowsl, qsl],
                            start=True, stop=False, tile_position=tpos,
                        )
                        nc.tensor.matmul(
                            out=spair[:, csl], lhsT=kB[rowsl, ksl],
                            rhs=qB[rowsl, qsl],
                            start=False, stop=not diag, tile_position=tpos,
                        )
                        if diag:
                            # causal mask: accumulate identity @ mask
                            nc.tensor.matmul(
                                out=spair[:, csl], lhsT=idnB,
                                rhs=maskAB[:, csl],
                                start=False, stop=True,
                                skip_group_check=True,
                            )
                    at = atp.tile(
                        [128, 2 * QB], BF16, tag="at", name=f"at{l}_{h4}_{qb}_{pr}"
                    )
                    nc.scalar.activation(out=at, in_=spair, func=AF.Exp, scale=SCALE)
                    ats.append(at)
                return ats

            def emit_attention(l, tiles, ao, post_sh0_hook, post_sh0_hook2):
                den = [
                    sm.tile([1, S], F32, tag=f"den{h4}", name=f"den{l}_{h4}")
                    for h4 in range(NH)
                ]
                recip = [
                    sm.tile([1, S], mybir.dt.float32r, tag=f"recip{h4}", name=f"recip{l}_{h4}")
                    for h4 in range(NH)
                ]
                pvp_stash = {}

                norm_queue = []

                def emit_avs(h4, qb, ats, gidx):
                    qsl = slice(qb * QB, (qb + 1) * QB)
                    nkc = 2 * (qb + 1)
                    pvp = ps.tile([128, QB], F32, tag="ps", name=f"pvp{l}_{h4}_{qb}")
                    for kc in range(nkc):
                        nc.tensor.matmul(
                            out=pvp[0:65, :],
                            lhsT=v_sb[:, kc, 72 * h4 : 72 * h4 + 65],
                            rhs=ats[kc // 2][:, (kc % 2) * QB : (kc % 2 + 1) * QB],
                            start=(kc == 0), stop=(kc == nkc - 1),
                        )
                    nc.scalar.copy(out=den[h4][:, qsl], in_=pvp[64:65, :])
                    pvp_stash[(h4, qb)] = pvp
                    if qb % 2 == 1:
                        sh = qb // 2
                        sl = slice(sh * SH, (sh + 1) * SH)
                        with nc.allow_low_precision(reason="recip f32r is full fp32"):
                            nc.vector.reciprocal(
                                out=recip[h4][:, sl], in_=den[h4][:, sl]
                            )
                        norm_queue.append((h4, sh, gidx))

                def emit_norm(h4, sh):
                    # bc/bcs/ao deferred one score-group after the divide so
                    # the PE never waits on the Pool-divide chain
                    sl = slice(sh * SH, (sh + 1) * SH)
                    bc = ps.tile([64, SH], F32, tag="ps", name=f"abc{l}_{h4}_{sh}")
                    nc.tensor.matmul(
                        out=bc, lhsT=onesEr, rhs=recip[h4][:, sl],
                        start=True, stop=True,
                    )
                    bcs = rt.tile([64, SH], BF16, tag="bcs", name=f"bcs{l}_{h4}_{sh}")
                    nc.vector.tensor_copy(out=bcs, in_=bc)
                    for qh in range(2):
                        qbb = 2 * sh + qh
                        nc.vector.tensor_tensor(
                            out=ao[h4 % 2][
                                64 * (h4 // 2) : 64 * (h4 // 2) + 64,
                                qbb * QB : (qbb + 1) * QB,
                            ],
                            in0=pvp_stash.pop((h4, qbb))[0:64, :],
                            in1=bcs[:, qh * QB : (qh + 1) * QB],
                            op=OP.mult,
                        )

                pend = None
                nsh0 = 0
                fired_at = None
                hook2_fired = False
                groups = [(h4, qb) for qb in range(NQB) for h4 in range(NH)]
                for gidx, (h4, qb) in enumerate(groups):
                    ats = emit_scores(l, h4, qb, tiles)
                    if pend is not None:
                        emit_avs(*pend, gidx)
                        while norm_queue and norm_queue[0][2] < gidx:
                            hh, ss, _ = norm_queue.pop(0)
                            emit_norm(hh, ss)
                            nsh0 += ss == 0
                    if nsh0 == NH and fired_at is None:
                        # all sh0 heads normalized: o-proj(sh0) overlaps the
                        # remaining big score groups
                        fired_at = gidx
                        post_sh0_hook()
                    elif fired_at is not None and gidx > fired_at and not hook2_fired:
                        # one group later: rms2(sh0) reduce (its residual/sq
                        # deps have drained behind the intervening scores)
                        hook2_fired = True
                        post_sh0_hook2()
                    pend = (h4, qb, ats)
                emit_avs(*pend, len(groups))
                while norm_queue:
                    hh, ss, _ = norm_queue.pop(0)
                    emit_norm(hh, ss)
                if fired_at is None:
                    post_sh0_hook()
                if not hook2_fired:
                    post_sh0_hook2()

            def oproj_half(l, ao, ow, sh):
                sl = slice(sh * SH, (sh + 1) * SH)
                for dco in range(2):
                    po = ps.tile([DC, SH], F32, tag="ps", name=f"po{l}_{sh}_{dco}")
                    for dci in range(2):
                        nc.tensor.matmul(
                            out=po, lhsT=ow[:, dci, dco * DC : (dco + 1) * DC],
                            rhs=ao[dci][:, sl], start=(dci == 0), stop=(dci == 1),
                        )
                    nc.vector.tensor_tensor(
                        out=x[dco][:, sl], in0=x[dco][:, sl], in1=po, op=OP.add
                    )

            def mlp_half(l, h2, tt, gw, uw, dw, sh, mid_hook=None):
                sl = slice(sh * SH, (sh + 1) * SH)
                for fc in range(8):
                    pg = ps.tile([FC, SH], F32, tag="ps", name=f"pg{l}_{sh}_{fc}")
                    pu = ps.tile([FC, SH], F32, tag="ps", name=f"pu{l}_{sh}_{fc}")
                    for dc in range(2):
                        nc.tensor.matmul(
                            out=pg, lhsT=gw[:, dc, fc * FC : (fc + 1) * FC],
                            rhs=h2[dc][:, sl], start=(dc == 0), stop=(dc == 1),
                        )
                    for dc in range(2):
                        nc.tensor.matmul(
                            out=pu, lhsT=uw[:, dc, fc * FC : (fc + 1) * FC],
                            rhs=h2[dc][:, sl], start=(dc == 0), stop=(dc == 1),
                        )
                    sg = rt.tile([FC, SH], BF16, tag="sg")
                    nc.scalar.activation(out=sg, in_=pg, func=AF.Silu, scale=1.0)
                    nc.vector.tensor_tensor(
                        out=tt[fc][:, sl], in0=sg, in1=pu, op=OP.mult
                    )
                if mid_hook is not None:
                    mid_hook()
                for dco in range(2):
                    pd = ps.tile([DC, SH], F32, tag="ps", name=f"pd{l}_{sh}_{dco}")
                    for fc in range(8):
                        nc.tensor.matmul(
                            out=pd, lhsT=dw[:, fc, dco * DC : (dco + 1) * DC],
                            rhs=tt[fc][:, sl], start=(fc == 0), stop=(fc == 7),
                        )
                    nc.vector.tensor_tensor(
                        out=x[dco][:, sl], in0=x[dco][:, sl], in1=pd, op=OP.add
                    )

            # =================== layers ===================
            # software-pipelined: rms1 of layer l+1 is emitted inside the
            # MLP of layer l (after each s-half's residual lands) so its
            # Act-engine chain hides behind MLP matmuls.
            h_cur, sq_cur = rms_alloc("h0_")
            for sh in range(2):
                rms_half(x, h_cur, sq_cur, "h0_", sh)
            pend_rmsB = []  # deferred part-B of the next layer's rms1
            for l in range(NL):
                for args in pend_rmsB:
                    rms_partB(*args)
                pend_rmsB = []
                h = h_cur
                tiles = (
                    qkp.tile([128, S], BF16, tag="qA", name=f"qA{l}"),
                    qkp.tile([128, S], BF16, tag="qB", name=f"qB{l}"),
                    qkp.tile([128, S], BF16, tag="kA", name=f"kA{l}"),
                    qkp.tile([128, S], BF16, tag="kB", name=f"kB{l}"),
                )
                emit_qkv_rope(l, h, wts["qkvw"], tiles)
                # o-proj(sh0) + rms2(sh0) are emitted inside the attention
                # stream (after all sh0 heads normalize) to hide their
                # latency behind the remaining score groups.
                h2, sq2 = rms_alloc(f"g{l}_")
                ow = wts["ow"]
                ao = [
                    aop.tile([128, S], BF16, tag=f"ao{c}", name=f"ao{l}_{c}")
                    for c in range(2)
                ]

                box = {}

                def post_sh0():
                    oproj_half(l, ao, ow, 0)

                def post_sh0b():
                    box["g0"] = rms_partA(x, sq2, f"g{l}_", 0)

                emit_attention(l, tiles, ao, post_sh0, post_sh0b)
                oproj_half(l, ao, ow, 1)
                rms_partB(x, h2, box["g0"], f"g{l}_", 0)
                gw, uw, dw = wts["gw"], wts["uw"], wts["dw"]
                # prefetch next layer's weights (overlaps MLP compute)
                if l + 1 < NL:
                    wts = load_weights(l + 1)
                ntag = f"h{l + 1}_" if l + 1 < NL else "f_"
                h_nxt, sq_nxt = rms_alloc(ntag)
                tt = [
                    tp.tile([FC, S], BF16, tag="t", name=f"t{l}_{i}")
                    for i in range(8)
                ]
                def a_g1():
                    box["g1"] = rms_partA(x, sq2, f"g{l}_", 1)

                def a_n0():
                    box["n0"] = rms_partA(x, sq_nxt, ntag, 0)

                mlp_half(l, h2, tt, gw, uw, dw, 0, mid_hook=a_g1)
                rms_partB(x, h2, box["g1"], f"g{l}_", 1)
                mlp_half(l, h2, tt, gw, uw, dw, 1, mid_hook=a_n0)
                rstd_n1 = rms_partA(x, sq_nxt, ntag, 1)
                pend_rmsB = [
                    (x, h_nxt, box["n0"], ntag, 0),
                    (x, h_nxt, rstd_n1, ntag, 1),
                ]
                h_cur = h_nxt

            def emit_lm_head(hf):
                vg_sizes = [512] * 15 + [320]
                voff = 0
                use_act = False
                for vg, vgs in enumerate(vg_sizes):
                    lmw = lmp.tile([DC, 2, 512], BF16, tag="lmw")
                    nc.sync.dma_start(
                        out=lmw[:, :, 0:vgs],
                        in_=lmh_d.ap()[:, :, voff : voff + vgs].rearrange("a p c -> p a c"),
                    )
                    nvc = (vgs + 127) // 128
                    lo = lop.tile([128, 4, S], BF16, tag="lo")
                    for vc in range(nvc):
                        m = min(128, vgs - vc * 128)
                        for sh in range(2):
                            sl = slice(sh * SH, (sh + 1) * SH)
                            pl = ps.tile([128, SH], F32, tag="ps")
                            for dc in range(2):
                                nc.tensor.matmul(
                                    out=pl[0:m, :],
                                    lhsT=lmw[:, dc, vc * 128 : vc * 128 + m],
                                    rhs=hf[dc][:, sl], start=(dc == 0), stop=(dc == 1),
                                )
                            if use_act:
                                nc.scalar.copy(out=lo[0:m, vc, sl], in_=pl[0:m, :])
                            else:
                                nc.vector.tensor_copy(out=lo[0:m, vc, sl], in_=pl[0:m, :])
                            use_act = not use_act
                    # one batched store per vocab group (vgs rows)
                    nfull = vgs // 128
                    nc.sync.dma_start(
                        out=out_d.ap()[voff : voff + nfull * 128, :].rearrange(
                            "(a p) s -> p a s", p=128
                        ),
                        in_=lo[:, 0:nfull, :],
                    )
                    if vgs % 128:
                        nc.sync.dma_start(
                            out=out_d.ap()[voff + nfull * 128 : voff + vgs, :],
                            in_=lo[0 : vgs % 128, nfull, :],
                        )
                    voff += vgs

            # =================== final norm + lm head ===================
            for args in pend_rmsB:
                rms_partB(*args)
            emit_lm_head(h_cur)

    nc.compile()
    _NC_CACHE = nc
    return nc


def _host_prep(inputs):
    """Build per-core in_maps from full inputs."""
    input_ids = np.asarray(inputs["input_ids"])
    embed = np.asarray(inputs["embed"], dtype=np.float32)
    ln1_w = np.asarray(inputs["ln1_w"], dtype=np.float32)
    qkv_w = np.asarray(inputs["qkv_w"], dtype=np.float32)
    o_w = np.asarray(inputs["o_w"], dtype=np.float32)
    ln2_w = np.asarray(inputs["ln2_w"], dtype=np.float32)
    gate_w = np.asarray(inputs["gate_w"], dtype=np.float32)
    up_w = np.asarray(inputs["up_w"], dtype=np.float32)
    down_w = np.asarray(inputs["down_w"], dtype=np.float32)
    norm_w = np.asarray(inputs["norm_w"], dtype=np.float32)
    lm_head_w = np.asarray(inputs["lm_head_w"], dtype=np.float32)
    bf16 = ml_dtypes.bfloat16

    # rope tables (transposed, padded 27->32, tiled x4 heads)
    inv_freq = 1.0 / (THETA ** (np.arange(0, HD, 2, dtype=np.float32) / HD))  # [27]
    t = np.arange(S, dtype=np.float32)
    freqs = np.outer(inv_freq, t)  # [27, S]
    cosh = np.cos(freqs).astype(np.float32)
    sinh = np.sin(freqs).astype(np.float32)

    def pad_tile(a):  # [27, S] -> [128, S]
        z = np.zeros((HP, S), np.float32)
        z[:HDH] = a
        return np.tile(z, (NH, 1))

    cos4 = pad_tile(cosh).astype(bf16)
    sin4p = pad_tile(sinh).astype(bf16)
    sin4n = pad_tile(-sinh).astype(bf16)

    # causal mask bias tiles [128 k, 256 q]: allowed k <= q
    r = np.arange(128)[:, None]
    c = np.arange(QB)[None, :]
    maskA = np.where(r <= c, 0.0, MASKVAL).astype(np.float32)
    maskB = np.where(r <= c - 128, 0.0, MASKVAL).astype(np.float32)
    maskAB = np.concatenate([maskA, maskB], axis=1).astype(bf16)

    idn = np.eye(128, dtype=np.float32)
    idnB = np.eye(128, dtype=np.float32).astype(bf16)

    # ---- weight packing ----
    # fold ln weights into qkv/gate/up; norm into lm_head
    qkvT = (qkv_w * ln1_w[:, None, :]).transpose(0, 2, 1)  # [NL, D(d), 3D(e)]
    gwT = (gate_w * ln2_w[:, None, :]).transpose(0, 2, 1)  # [NL, D, FF]
    uwT = (up_w * ln2_w[:, None, :]).transpose(0, 2, 1)  # [NL, D, FF]
    owT = o_w.transpose(0, 2, 1)
    dwT = down_w.transpose(0, 2, 1)  # [NL, FF, D]
    lmT = (lm_head_w * norm_w[None, :]).T  # [D, V]

    # q/k packed with lo/hi split, 32-padded: cols [qA(128) qB(128) kA kB v(256)]
    qkvw_packed = np.zeros((NL, D, 768), np.float32)
    for h in range(NH):
        qkvw_packed[:, :, HP * h : HP * h + HDH] = qkvT[:, :, 54 * h : 54 * h + HDH]
        qkvw_packed[:, :, 128 + HP * h : 128 + HP * h + HDH] = qkvT[:, :, 54 * h + HDH : 54 * h + HD]
        qkvw_packed[:, :, 256 + HP * h : 256 + HP * h + HDH] = qkvT[:, :, D + 54 * h : D + 54 * h + HDH]
        qkvw_packed[:, :, 384 + HP * h : 384 + HP * h + HDH] = qkvT[:, :, D + 54 * h + HDH : D + 54 * h + HD]
    qkvw_packed[:, :, 512 : 512 + D] = qkvT[:, :, 2 * D : 3 * D]  # v natural
    qkvw = qkvw_packed.reshape(NL, 2, DC, 768).astype(bf16)

    ow = np.zeros((NL, 2, 128, D), np.float32)
    for hh in range(NH):
        cc, j = hh % 2, hh // 2
        ow[:, cc, 64 * j : 64 * j + HD, :] = owT[:, 54 * hh : 54 * (hh + 1), :]
    ow = ow.astype(bf16)
    gwp = gwT.reshape(NL, 2, DC, FF).astype(bf16)
    uwp = uwT.reshape(NL, 2, DC, FF).astype(bf16)
    dwp = dwT.reshape(NL, 8, FC, D).astype(bf16)

    common = dict(
        embed=embed, cos4=cos4, sin4p=sin4p, sin4n=sin4n,
        maskAB=maskAB, idn=idn, idnB=idnB,
        onesEr=np.ones((1, 64), np.float32),
        qkvw=qkvw, ow=ow, gw=gwp, uw=uwp, dw=dwp,
    )
    in_maps = []
    for core in range(NCORES):
        b = core // VSPLIT
        vs = core % VSPLIT
        m = dict(common)
        m["ids"] = input_ids[b].astype(np.int32).reshape(S, 1)
        m["lmh"] = np.ascontiguousarray(
            lmT[:, vs * VS : (vs + 1) * VS].reshape(2, DC, VS)
        ).astype(bf16)
        in_maps.append(m)
    return in_maps


def kernel(**inputs) -> np.ndarray:
    nc = build_nc()
    in_maps = _host_prep(inputs)
    res = bass_utils.run_bass_kernel_spmd(nc, in_maps, core_ids=list(range(NCORES)))
    out = np.empty((B, S, V), np.float32)
    for core in range(NCORES):
        b = core // VSPLIT
        vs = core % VSPLIT
        out[b, :, vs * VS : (vs + 1) * VS] = (
            np.asarray(res.results[core]["logitsT"]).astype(np.float32).T
        )
    return out
